# revision 22
# baseline (speedup 1.0000x reference)
"""Trainium2 Bass kernel for nn_DigitConvolutionalModel (3x3 valid conv + 3-layer MLP).

Strategy
--------
The 3x3 "valid" cross-correlation is linear in x, so it is folded on the host
into the first MLP weight:  conv(x).reshape(B, 676) @ w1  ==  x @ weff  with
weff[784, 256] built from conv_w and w1.  The device then runs a pure 3-layer
MLP:

    out = relu(relu(x @ weff + b1) @ w2 + b2) @ w3 + b3

Data-parallel over the batch across 8 NeuronCores (8192 rows per core).
On-chip dataflow is feature-major ([features, batch] tiles) so the contraction
dim of every matmul lands on SBUF partitions with zero on-chip transposes; the
host pre-tiles each x shard into the exact SBUF layout so every device DMA is
a fully contiguous HBM read, and transposes the [10, batch] result back.

The matmul path runs in float16 (fp32 PSUM accumulation): x and the weights
are rounded to fp16 on the host.  Measured end-to-end relative error vs the
fp32 reference is ~5e-4 (fp16's 10 mantissa bits; activations are O(10) so
there is no range risk).  fp16 halves DMA bytes and enables full-rate PE
matmuls with fast weight loads.

Schedule: a software pipeline L1(n) | L2(n-1) | L3(n-2) with explicit PE
issue-order edges so the in-order PE queue never waits on ACT/DVE epilogues,
plus a short burst of dummy matmuls at kernel start to lift the PE's HAM
clock gate to full rate while the first DMAs are in flight.
"""

import numpy as np

import concourse.bass as bass
import concourse.mybir as mybir
import concourse.tile as tile
from bass_rust import add_dep_helper
from concourse import bacc
from concourse.bass_utils import run_bass_kernel_spmd

N_CORES = 8
B = 65536
BS = B // N_CORES          # 8192 batch rows per core
KIN = 784                  # input features (28*28)
KC, KCH = 7, 112           # layer-1 contraction chunks: 7 x 112 = 784
H1, H2, NOUT = 256, 128, 10
NB = 512                   # batch tile (matmul free dim = one PSUM bank of fp32)
NITER = BS // NB           # 16

F32 = mybir.dt.float32
F16 = mybir.dt.float16
RELU = mybir.ActivationFunctionType.Relu


def build_program():
    nc = bacc.Bacc(
        "TRN2", target_bir_lowering=False, debug=False, num_devices=N_CORES
    )
    # all tensors arrive pre-tiled from the host in the exact SBUF layout so
    # every DMA reads DRAM fully contiguously (max HBM burst efficiency)
    xt_d = nc.dram_tensor("xt", [NITER, KCH, KC, NB], F16,
                          kind="ExternalInput").ap()
    weff_d = nc.dram_tensor("weff", [KCH, KC, H1], F16,
                            kind="ExternalInput").ap()
    w2_d = nc.dram_tensor("w2", [128, 2, H2], F16, kind="ExternalInput").ap()
    w3_d = nc.dram_tensor("w3", [H2, NOUT], F16, kind="ExternalInput").ap()
    b1_d = nc.dram_tensor("b1", [128, 2], F32, kind="ExternalInput").ap()
    b2_d = nc.dram_tensor("b2", [128, 1], F32, kind="ExternalInput").ap()
    out_d = nc.dram_tensor("out", [NITER, NOUT, NB], F32,
                           kind="ExternalOutput").ap()

    with tile.TileContext(nc) as tc:
        with (
            tc.tile_pool(name="w", bufs=1) as wp,
            tc.tile_pool(name="x", bufs=8) as xp,
            tc.tile_pool(name="h", bufs=4) as hp,
            tc.tile_pool(name="o", bufs=2) as op,
            tc.tile_pool(name="ps", bufs=2, space=bass.MemorySpace.PSUM) as pp,
        ):
            # HAM warmup: dummy matmuls on memset data raise the PE clock to
            # 8/8 while the first x/weff DMAs are still in flight
            warm = wp.tile([KCH, NB], F16, tag="warm")
            nc.vector.memset(warm[:], 0.0)
            pw = pp.tile([128, NB], F32, tag="p1_0")
            last_mm = None  # previous PE instruction, for ordering edges
            for _ in range(8):
                mm = nc.tensor.matmul(pw[:], warm[:, 0:128], warm[:],
                                      start=True, stop=True)
                if last_mm is not None:
                    add_dep_helper(mm.ins, last_mm.ins, sync=False,
                                   reason="PE issue order")
                last_mm = mm

            # weights go on the gpsimd queue so the first x tile (sync queue)
            # lands concurrently
            weff_t = wp.tile([KCH, KC, H1], F16, tag="weff")
            nc.gpsimd.dma_start(weff_t[:], weff_d[:])
            w2_t = wp.tile([128, 2, H2], F16, tag="w2")
            nc.gpsimd.dma_start(w2_t[:], w2_d[:])
            w3_t = wp.tile([H2, NOUT], F16, tag="w3")
            nc.gpsimd.dma_start(w3_t[:], w3_d[:])
            b1_t = wp.tile([128, 2], F32, tag="b1")
            nc.gpsimd.dma_start(b1_t[:], b1_d[:])
            b2_t = wp.tile([128, 1], F32, tag="b2")
            nc.gpsimd.dma_start(b2_t[:], b2_d[:])

            # software pipeline: L1(n) | L2(n-1) | L3(n-2) so the in-order PE
            # queue never waits on the ACT/DVE epilogues of the same iteration
            h1_hist = {}
            h2_hist = {}
            for n in range(NITER + 2):
                if n < NITER:
                    xt = xp.tile([KCH, KC, NB], F16, tag="x")
                    if n == 0:
                        nc.sync.dma_start(xt[:, 0:2, :], xt_d[n][:, 0:2, :])
                        nc.sync.dma_start(xt[:, 2:4, :], xt_d[n][:, 2:4, :])
                        nc.sync.dma_start(xt[:, 4:KC, :], xt_d[n][:, 4:KC, :])
                    else:
                        nc.sync.dma_start(xt[:], xt_d[n])
                    h1s = []
                    for m in range(2):
                        p1 = pp.tile([128, NB], F32, tag=f"p1_{m}")
                        for k in range(KC):
                            mm = nc.tensor.matmul(
                                p1[:],
                                weff_t[:, k, m * 128:(m + 1) * 128],
                                xt[:, k, :],
                                start=(k == 0),
                                stop=(k == KC - 1),
                            )
                            if last_mm is not None:
                                add_dep_helper(mm.ins, last_mm.ins, sync=False,
                                               reason="PE issue order")
                            last_mm = mm
                        h1 = hp.tile([128, NB], F16, tag=f"h1_{m}")
                        nc.scalar.activation(
                            h1[:], p1[:], RELU, bias=b1_t[:, m:m + 1]
                        )
                        h1s.append(h1)
                        if m == 0 and 0 <= n - 2 < NITER:
                            # L3(n-2) sandwiched mid-L1 so its short drain
                            # overlaps the m1 weight loads
                            h2 = h2_hist.pop(n - 2)
                            p3 = pp.tile([NOUT, NB], F32, tag="p3")
                            mm = nc.tensor.matmul(
                                p3[:], w3_t[:], h2[:], start=True, stop=True,
                            )
                            add_dep_helper(mm.ins, last_mm.ins, sync=False,
                                           reason="PE issue order")
                            last_mm = mm
                            ot = op.tile([NOUT, NB], F32, tag="ot")
                            nc.vector.tensor_copy(ot[:], p3[:])
                            nc.gpsimd.dma_start(out_d[n - 2], ot[:])
                    h1_hist[n] = h1s
                if 0 <= n - 1 < NITER:
                    h1s = h1_hist.pop(n - 1)
                    p2 = pp.tile([128, NB], F32, tag="p2")
                    for k in range(2):
                        mm = nc.tensor.matmul(
                            p2[:],
                            w2_t[:, k, :],
                            h1s[k][:],
                            start=(k == 0),
                            stop=(k == 1),
                        )
                        add_dep_helper(mm.ins, last_mm.ins, sync=False,
                                       reason="PE issue order")
                        last_mm = mm
                    h2 = hp.tile([128, NB], F16, tag="h2")
                    nc.vector.tensor_scalar(
                        h2[:], p2[:], b2_t[:, 0:1], 0.0,
                        mybir.AluOpType.add, mybir.AluOpType.max,
                    )
                    h2_hist[n - 1] = h2
                if n >= NITER and 0 <= n - 2 < NITER:
                    # pipeline drain: last two L3s have no later L1 to hide in
                    h2 = h2_hist.pop(n - 2)
                    p3 = pp.tile([NOUT, NB], F32, tag="p3")
                    mm = nc.tensor.matmul(
                        p3[:], w3_t[:], h2[:], start=True, stop=True,
                    )
                    add_dep_helper(mm.ins, last_mm.ins, sync=False,
                                   reason="PE issue order")
                    last_mm = mm
                    ot = op.tile([NOUT, NB], F32, tag="ot")
                    nc.vector.tensor_copy(ot[:], p3[:])
                    nc.gpsimd.dma_start(out_d[n - 2], ot[:])

    nc.compile()
    return nc


_NC = None


def _get_program():
    global _NC
    if _NC is None:
        _NC = build_program()
    return _NC


def make_in_maps(x, conv_w, w1, b1, w2, b2, w3, b3):
    """Host-side prep: fold conv into w1, pre-tile everything into the exact
    on-chip layout so device DMAs are fully contiguous."""
    conv_w = np.asarray(conv_w, np.float64)
    w1r = np.asarray(w1, np.float64).reshape(26, 26, H1)
    weff = np.zeros((28, 28, H1), np.float64)
    for u in range(3):
        for v in range(3):
            weff[u:u + 26, v:v + 26, :] += conv_w[u, v] * w1r
    weff = weff.reshape(KIN, H1).astype(np.float16)
    # [784, 256] -> [112, 7, 256]
    weff_d = np.ascontiguousarray(weff.reshape(KC, KCH, H1).transpose(1, 0, 2))
    # [256, 128] -> [128, 2, 128]
    w2_d = np.ascontiguousarray(
        np.asarray(w2, np.float16).reshape(2, 128, H2).transpose(1, 0, 2))

    b1d = np.ascontiguousarray(np.asarray(b1, np.float32).reshape(2, 128).T)
    b2d = np.ascontiguousarray(np.asarray(b2, np.float32).reshape(128, 1))
    w3c = np.ascontiguousarray(np.asarray(w3, np.float16))

    x = np.asarray(x, np.float16)
    in_maps = []
    for c in range(N_CORES):
        # [8192, 784] -> feature-major tiles [NITER, 112, 7, 512]
        xs = x[c * BS:(c + 1) * BS].T  # [784, 8192] view
        xs = np.ascontiguousarray(
            xs.reshape(KC, KCH, NITER, NB).transpose(2, 1, 0, 3))
        in_maps.append({
            "xt": xs, "weff": weff_d, "w2": w2_d, "w3": w3c,
            "b1": b1d, "b2": b2d,
        })
    return in_maps


def run(x, conv_w, w1, b1, w2, b2, w3, b3, trace=False):
    nc = _get_program()
    in_maps = make_in_maps(x, conv_w, w1, b1, w2, b2, w3, b3)
    br = run_bass_kernel_spmd(nc, in_maps, core_ids=list(range(N_CORES)),
                              trace=trace)
    out = np.empty((B, NOUT), np.float32)
    for c in range(N_CORES):
        # [NITER, 10, 512] -> [8192, 10]
        r = br.results[c]["out"]
        out[c * BS:(c + 1) * BS] = r.transpose(0, 2, 1).reshape(BS, NOUT)
    out += np.asarray(b3, np.float32)[None, :]
    return out, br


def kernel(x, conv_w, w1, b1, w2, b2, w3, b3):
    out, _ = run(x, conv_w, w1, b1, w2, b2, w3, b3)
    return out



# revision 23
# speedup vs baseline: 1.0071x; 1.0071x over previous
"""Trainium2 Bass kernel for nn_DigitConvolutionalModel (3x3 valid conv + 3-layer MLP).

Strategy
--------
The 3x3 "valid" cross-correlation is linear in x, so it is folded on the host
into the first MLP weight:  conv(x).reshape(B, 676) @ w1  ==  x @ weff  with
weff[784, 256] built from conv_w and w1.  The device then runs a pure 3-layer
MLP:

    out = relu(relu(x @ weff + b1) @ w2 + b2) @ w3 + b3

Data-parallel over the batch across 8 NeuronCores (8192 rows per core).
On-chip dataflow is feature-major ([features, batch] tiles) so the contraction
dim of every matmul lands on SBUF partitions with zero on-chip transposes; the
host pre-tiles each x shard into the exact SBUF layout so every device DMA is
a fully contiguous HBM read, and transposes the [10, batch] result back.

The matmul path runs in float16 (fp32 PSUM accumulation): x and the weights
are rounded to fp16 on the host.  Measured end-to-end relative error vs the
fp32 reference is ~5e-4 (fp16's 10 mantissa bits; activations are O(10) so
there is no range risk).  fp16 halves DMA bytes and enables full-rate PE
matmuls with fast weight loads.

Schedule: a software pipeline L1(n) | L2(n-1) | L3(n-2) with explicit PE
issue-order edges so the in-order PE queue never waits on ACT/DVE epilogues,
plus a short burst of dummy matmuls at kernel start to lift the PE's HAM
clock gate to full rate while the first DMAs are in flight.
"""

import numpy as np

import concourse.bass as bass
import concourse.mybir as mybir
import concourse.tile as tile
from bass_rust import add_dep_helper
from concourse import bacc
from concourse.bass_utils import run_bass_kernel_spmd

N_CORES = 8
B = 65536
BS = B // N_CORES          # 8192 batch rows per core
KIN = 784                  # input features (28*28)
KC, KCH = 7, 112           # layer-1 contraction chunks: 7 x 112 = 784
H1, H2, NOUT = 256, 128, 10
NB = 512                   # batch tile (matmul free dim = one PSUM bank of fp32)
NITER = BS // NB           # 16

F32 = mybir.dt.float32
F16 = mybir.dt.float16
RELU = mybir.ActivationFunctionType.Relu


def build_program():
    nc = bacc.Bacc(
        "TRN2", target_bir_lowering=False, debug=False, num_devices=N_CORES
    )
    # all tensors arrive pre-tiled from the host in the exact SBUF layout so
    # every DMA reads DRAM fully contiguously (max HBM burst efficiency)
    xt_d = nc.dram_tensor("xt", [NITER, KCH, KC, NB], F16,
                          kind="ExternalInput").ap()
    weff_d = nc.dram_tensor("weff", [KCH, KC, H1], F16,
                            kind="ExternalInput").ap()
    w2_d = nc.dram_tensor("w2", [128, 2, H2], F16, kind="ExternalInput").ap()
    w3_d = nc.dram_tensor("w3", [H2, NOUT], F16, kind="ExternalInput").ap()
    b1_d = nc.dram_tensor("b1", [128, 2], F32, kind="ExternalInput").ap()
    b2_d = nc.dram_tensor("b2", [128, 1], F32, kind="ExternalInput").ap()
    out_d = nc.dram_tensor("out", [NITER, NOUT, NB], F32,
                           kind="ExternalOutput").ap()

    with tile.TileContext(nc) as tc:
        with (
            tc.tile_pool(name="w", bufs=1) as wp,
            tc.tile_pool(name="x", bufs=8) as xp,
            tc.tile_pool(name="h", bufs=4) as hp,
            tc.tile_pool(name="o", bufs=2) as op,
            tc.tile_pool(name="ps", bufs=2, space=bass.MemorySpace.PSUM) as pp,
        ):
            # HAM warmup: dummy matmuls on memset data raise the PE clock to
            # 8/8 while the first x/weff DMAs are still in flight
            warm = wp.tile([KCH, NB], F16, tag="warm")
            nc.vector.memset(warm[:], 0.0)
            pw = pp.tile([128, NB], F32, tag="p1_0")
            last_mm = None  # previous PE instruction, for ordering edges
            for _ in range(10):
                mm = nc.tensor.matmul(pw[:], warm[:, 0:128], warm[:],
                                      start=True, stop=True)
                if last_mm is not None:
                    add_dep_helper(mm.ins, last_mm.ins, sync=False,
                                   reason="PE issue order")
                last_mm = mm

            # weights go on the gpsimd queue so the first x tile (sync queue)
            # lands concurrently
            weff_t = wp.tile([KCH, KC, H1], F16, tag="weff")
            nc.gpsimd.dma_start(weff_t[:], weff_d[:])
            w2_t = wp.tile([128, 2, H2], F16, tag="w2")
            nc.gpsimd.dma_start(w2_t[:], w2_d[:])
            w3_t = wp.tile([H2, NOUT], F16, tag="w3")
            nc.gpsimd.dma_start(w3_t[:], w3_d[:])
            b1_t = wp.tile([128, 2], F32, tag="b1")
            nc.gpsimd.dma_start(b1_t[:], b1_d[:])
            b2_t = wp.tile([128, 1], F32, tag="b2")
            nc.gpsimd.dma_start(b2_t[:], b2_d[:])

            # software pipeline: L1(n) | L2(n-1) | L3(n-2) so the in-order PE
            # queue never waits on the ACT/DVE epilogues of the same iteration
            h1_hist = {}
            h2_hist = {}
            for n in range(NITER + 2):
                if n < NITER:
                    xt = xp.tile([KCH, KC, NB], F16, tag="x")
                    if n == 0:
                        nc.sync.dma_start(xt[:, 0:2, :], xt_d[n][:, 0:2, :])
                        nc.sync.dma_start(xt[:, 2:4, :], xt_d[n][:, 2:4, :])
                        nc.sync.dma_start(xt[:, 4:KC, :], xt_d[n][:, 4:KC, :])
                    else:
                        nc.sync.dma_start(xt[:], xt_d[n])
                    h1s = []
                    for m in range(2):
                        p1 = pp.tile([128, NB], F32, tag=f"p1_{m}")
                        for k in range(KC):
                            mm = nc.tensor.matmul(
                                p1[:],
                                weff_t[:, k, m * 128:(m + 1) * 128],
                                xt[:, k, :],
                                start=(k == 0),
                                stop=(k == KC - 1),
                            )
                            if last_mm is not None:
                                add_dep_helper(mm.ins, last_mm.ins, sync=False,
                                               reason="PE issue order")
                            last_mm = mm
                        h1 = hp.tile([128, NB], F16, tag=f"h1_{m}")
                        nc.scalar.activation(
                            h1[:], p1[:], RELU, bias=b1_t[:, m:m + 1]
                        )
                        h1s.append(h1)
                    h1_hist[n] = h1s
                if 0 <= n - 1 < NITER:
                    h1s = h1_hist.pop(n - 1)
                    p2 = pp.tile([128, NB], F32, tag="p2")
                    for k in range(2):
                        mm = nc.tensor.matmul(
                            p2[:],
                            w2_t[:, k, :],
                            h1s[k][:],
                            start=(k == 0),
                            stop=(k == 1),
                        )
                        add_dep_helper(mm.ins, last_mm.ins, sync=False,
                                       reason="PE issue order")
                        last_mm = mm
                    h2 = hp.tile([128, NB], F16, tag="h2")
                    nc.vector.tensor_scalar(
                        h2[:], p2[:], b2_t[:, 0:1], 0.0,
                        mybir.AluOpType.add, mybir.AluOpType.max,
                    )
                    h2_hist[n - 1] = h2
                if 0 <= n - 2 < NITER:
                    h2 = h2_hist.pop(n - 2)
                    p3 = pp.tile([NOUT, NB], F32, tag="p3")
                    mm = nc.tensor.matmul(
                        p3[:], w3_t[:], h2[:], start=True, stop=True,
                    )
                    add_dep_helper(mm.ins, last_mm.ins, sync=False,
                                   reason="PE issue order")
                    last_mm = mm
                    ot = op.tile([NOUT, NB], F32, tag="ot")
                    nc.vector.tensor_copy(ot[:], p3[:])
                    nc.gpsimd.dma_start(out_d[n - 2], ot[:])

    nc.compile()
    return nc


_NC = None


def _get_program():
    global _NC
    if _NC is None:
        _NC = build_program()
    return _NC


def make_in_maps(x, conv_w, w1, b1, w2, b2, w3, b3):
    """Host-side prep: fold conv into w1, pre-tile everything into the exact
    on-chip layout so device DMAs are fully contiguous."""
    conv_w = np.asarray(conv_w, np.float64)
    w1r = np.asarray(w1, np.float64).reshape(26, 26, H1)
    weff = np.zeros((28, 28, H1), np.float64)
    for u in range(3):
        for v in range(3):
            weff[u:u + 26, v:v + 26, :] += conv_w[u, v] * w1r
    weff = weff.reshape(KIN, H1).astype(np.float16)
    # [784, 256] -> [112, 7, 256]
    weff_d = np.ascontiguousarray(weff.reshape(KC, KCH, H1).transpose(1, 0, 2))
    # [256, 128] -> [128, 2, 128]
    w2_d = np.ascontiguousarray(
        np.asarray(w2, np.float16).reshape(2, 128, H2).transpose(1, 0, 2))

    b1d = np.ascontiguousarray(np.asarray(b1, np.float32).reshape(2, 128).T)
    b2d = np.ascontiguousarray(np.asarray(b2, np.float32).reshape(128, 1))
    w3c = np.ascontiguousarray(np.asarray(w3, np.float16))

    x = np.asarray(x, np.float16)
    in_maps = []
    for c in range(N_CORES):
        # [8192, 784] -> feature-major tiles [NITER, 112, 7, 512]
        xs = x[c * BS:(c + 1) * BS].T  # [784, 8192] view
        xs = np.ascontiguousarray(
            xs.reshape(KC, KCH, NITER, NB).transpose(2, 1, 0, 3))
        in_maps.append({
            "xt": xs, "weff": weff_d, "w2": w2_d, "w3": w3c,
            "b1": b1d, "b2": b2d,
        })
    return in_maps


def run(x, conv_w, w1, b1, w2, b2, w3, b3, trace=False):
    nc = _get_program()
    in_maps = make_in_maps(x, conv_w, w1, b1, w2, b2, w3, b3)
    br = run_bass_kernel_spmd(nc, in_maps, core_ids=list(range(N_CORES)),
                              trace=trace)
    out = np.empty((B, NOUT), np.float32)
    for c in range(N_CORES):
        # [NITER, 10, 512] -> [8192, 10]
        r = br.results[c]["out"]
        out[c * BS:(c + 1) * BS] = r.transpose(0, 2, 1).reshape(BS, NOUT)
    out += np.asarray(b3, np.float32)[None, :]
    return out, br


def kernel(x, conv_w, w1, b1, w2, b2, w3, b3):
    out, _ = run(x, conv_w, w1, b1, w2, b2, w3, b3)
    return out



# revision 25
# speedup vs baseline: 1.0170x; 1.0098x over previous
"""Trainium2 Bass kernel for nn_DigitConvolutionalModel (3x3 valid conv + 3-layer MLP).

Strategy
--------
The 3x3 "valid" cross-correlation is linear in x, so it is folded on the host
into the first MLP weight:  conv(x).reshape(B, 676) @ w1  ==  x @ weff  with
weff[784, 256] built from conv_w and w1.  The device then runs a pure 3-layer
MLP:

    out = relu(relu(x @ weff + b1) @ w2 + b2) @ w3 + b3

Data-parallel over the batch across 8 NeuronCores (8192 rows per core).
On-chip dataflow is feature-major ([features, batch] tiles) so the contraction
dim of every matmul lands on SBUF partitions with zero on-chip transposes; the
host pre-tiles each x shard into the exact SBUF layout so every device DMA is
a fully contiguous HBM read, and transposes the [10, batch] result back.

The matmul path runs in float16 (fp32 PSUM accumulation): x and the weights
are rounded to fp16 on the host.  Measured end-to-end relative error vs the
fp32 reference is ~5e-4 (fp16's 10 mantissa bits; activations are O(10) so
there is no range risk).  fp16 halves DMA bytes and enables full-rate PE
matmuls with fast weight loads.

Schedule: a software pipeline L1(n) | L2(n-1) | L3(n-2) with explicit PE
issue-order edges so the in-order PE queue never waits on ACT/DVE epilogues,
plus a short burst of dummy matmuls at kernel start to lift the PE's HAM
clock gate to full rate while the first DMAs are in flight.
"""

import numpy as np

import concourse.bass as bass
import concourse.mybir as mybir
import concourse.tile as tile
from bass_rust import add_dep_helper
from concourse import bacc
from concourse.bass_utils import run_bass_kernel_spmd

N_CORES = 8
B = 65536
BS = B // N_CORES          # 8192 batch rows per core
KIN = 784                  # input features (28*28)
KC, KCH = 7, 112           # layer-1 contraction chunks: 7 x 112 = 784
H1, H2, NOUT = 256, 128, 10
NB = 512                   # batch tile (matmul free dim = one PSUM bank of fp32)
NITER = BS // NB           # 16

F32 = mybir.dt.float32
F16 = mybir.dt.float16
RELU = mybir.ActivationFunctionType.Relu


def build_program():
    nc = bacc.Bacc(
        "TRN2", target_bir_lowering=False, debug=False, num_devices=N_CORES
    )
    # all tensors arrive pre-tiled from the host in the exact SBUF layout so
    # every DMA reads DRAM fully contiguously (max HBM burst efficiency)
    xt_d = nc.dram_tensor("xt", [NITER, KCH, KC, NB], F16,
                          kind="ExternalInput").ap()
    weff_d = nc.dram_tensor("weff", [KCH, KC, H1], F16,
                            kind="ExternalInput").ap()
    w2_d = nc.dram_tensor("w2", [128, 2, H2], F16, kind="ExternalInput").ap()
    w3_d = nc.dram_tensor("w3", [H2, NOUT], F16, kind="ExternalInput").ap()
    b1_d = nc.dram_tensor("b1", [128, 2], F32, kind="ExternalInput").ap()
    b2_d = nc.dram_tensor("b2", [128, 1], F32, kind="ExternalInput").ap()
    out_d = nc.dram_tensor("out", [NITER, NOUT, NB], F32,
                           kind="ExternalOutput").ap()

    with tile.TileContext(nc) as tc:
        with (
            tc.tile_pool(name="w", bufs=1) as wp,
            tc.tile_pool(name="x", bufs=8) as xp,
            tc.tile_pool(name="h", bufs=4) as hp,
            tc.tile_pool(name="o", bufs=2) as op,
            tc.tile_pool(name="ps", bufs=2, space=bass.MemorySpace.PSUM) as pp,
        ):
            # HAM warmup: dummy matmuls raise the PE clock to 8/8 while the
            # first x/weff DMAs are in flight; operand data is never used, so
            # the tile is deliberately left uninitialized (no memset to wait on)
            warm = wp.tile([KCH, NB], F16, tag="warm")
            # one-column memset: cheapest possible writer so Tile allocates the
            # tile; the matmul operand values themselves are never consumed
            nc.gpsimd.memset(warm[:, 0:1], 0.0)
            pw = pp.tile([128, NB], F32, tag="p1_0")
            last_mm = None  # previous PE instruction, for ordering edges
            for _ in range(8):
                mm = nc.tensor.matmul(pw[:], warm[:, 0:128], warm[:],
                                      start=True, stop=True)
                if last_mm is not None:
                    add_dep_helper(mm.ins, last_mm.ins, sync=False,
                                   reason="PE issue order")
                last_mm = mm

            # weights go on the gpsimd queue so the first x tile (sync queue)
            # lands concurrently
            weff_t = wp.tile([KCH, KC, H1], F16, tag="weff")
            nc.gpsimd.dma_start(weff_t[:], weff_d[:])
            w2_t = wp.tile([128, 2, H2], F16, tag="w2")
            nc.gpsimd.dma_start(w2_t[:], w2_d[:])
            w3_t = wp.tile([H2, NOUT], F16, tag="w3")
            nc.gpsimd.dma_start(w3_t[:], w3_d[:])
            b1_t = wp.tile([128, 2], F32, tag="b1")
            nc.gpsimd.dma_start(b1_t[:], b1_d[:])
            b2_t = wp.tile([128, 1], F32, tag="b2")
            nc.gpsimd.dma_start(b2_t[:], b2_d[:])

            # software pipeline: L1(n) | L2(n-1) | L3(n-2) so the in-order PE
            # queue never waits on the ACT/DVE epilogues of the same iteration
            h1_hist = {}
            h2_hist = {}
            for n in range(NITER + 2):
                if n < NITER:
                    xt = xp.tile([KCH, KC, NB], F16, tag="x")
                    if n == 0:
                        nc.sync.dma_start(xt[:, 0:2, :], xt_d[n][:, 0:2, :])
                        nc.sync.dma_start(xt[:, 2:4, :], xt_d[n][:, 2:4, :])
                        nc.sync.dma_start(xt[:, 4:KC, :], xt_d[n][:, 4:KC, :])
                    else:
                        nc.sync.dma_start(xt[:], xt_d[n])
                    h1s = []
                    for m in range(2):
                        p1 = pp.tile([128, NB], F32, tag=f"p1_{m}")
                        for k in range(KC):
                            mm = nc.tensor.matmul(
                                p1[:],
                                weff_t[:, k, m * 128:(m + 1) * 128],
                                xt[:, k, :],
                                start=(k == 0),
                                stop=(k == KC - 1),
                            )
                            if last_mm is not None:
                                add_dep_helper(mm.ins, last_mm.ins, sync=False,
                                               reason="PE issue order")
                            last_mm = mm
                        h1 = hp.tile([128, NB], F16, tag=f"h1_{m}")
                        nc.scalar.activation(
                            h1[:], p1[:], RELU, bias=b1_t[:, m:m + 1]
                        )
                        h1s.append(h1)
                    h1_hist[n] = h1s
                if 0 <= n - 1 < NITER:
                    h1s = h1_hist.pop(n - 1)
                    p2 = pp.tile([128, NB], F32, tag="p2")
                    for k in range(2):
                        mm = nc.tensor.matmul(
                            p2[:],
                            w2_t[:, k, :],
                            h1s[k][:],
                            start=(k == 0),
                            stop=(k == 1),
                        )
                        add_dep_helper(mm.ins, last_mm.ins, sync=False,
                                       reason="PE issue order")
                        last_mm = mm
                    h2 = hp.tile([128, NB], F16, tag="h2")
                    nc.vector.tensor_scalar(
                        h2[:], p2[:], b2_t[:, 0:1], 0.0,
                        mybir.AluOpType.add, mybir.AluOpType.max,
                    )
                    h2_hist[n - 1] = h2
                if 0 <= n - 2 < NITER:
                    h2 = h2_hist.pop(n - 2)
                    p3 = pp.tile([NOUT, NB], F32, tag="p3")
                    mm = nc.tensor.matmul(
                        p3[:], w3_t[:], h2[:], start=True, stop=True,
                    )
                    add_dep_helper(mm.ins, last_mm.ins, sync=False,
                                   reason="PE issue order")
                    last_mm = mm
                    ot = op.tile([NOUT, NB], F32, tag="ot")
                    nc.vector.tensor_copy(ot[:], p3[:])
                    nc.gpsimd.dma_start(out_d[n - 2], ot[:])

    nc.compile()
    return nc


_NC = None


def _get_program():
    global _NC
    if _NC is None:
        _NC = build_program()
    return _NC


def make_in_maps(x, conv_w, w1, b1, w2, b2, w3, b3):
    """Host-side prep: fold conv into w1, pre-tile everything into the exact
    on-chip layout so device DMAs are fully contiguous."""
    conv_w = np.asarray(conv_w, np.float64)
    w1r = np.asarray(w1, np.float64).reshape(26, 26, H1)
    weff = np.zeros((28, 28, H1), np.float64)
    for u in range(3):
        for v in range(3):
            weff[u:u + 26, v:v + 26, :] += conv_w[u, v] * w1r
    weff = weff.reshape(KIN, H1).astype(np.float16)
    # [784, 256] -> [112, 7, 256]
    weff_d = np.ascontiguousarray(weff.reshape(KC, KCH, H1).transpose(1, 0, 2))
    # [256, 128] -> [128, 2, 128]
    w2_d = np.ascontiguousarray(
        np.asarray(w2, np.float16).reshape(2, 128, H2).transpose(1, 0, 2))

    b1d = np.ascontiguousarray(np.asarray(b1, np.float32).reshape(2, 128).T)
    b2d = np.ascontiguousarray(np.asarray(b2, np.float32).reshape(128, 1))
    w3c = np.ascontiguousarray(np.asarray(w3, np.float16))

    x = np.asarray(x, np.float16)
    in_maps = []
    for c in range(N_CORES):
        # [8192, 784] -> feature-major tiles [NITER, 112, 7, 512]
        xs = x[c * BS:(c + 1) * BS].T  # [784, 8192] view
        xs = np.ascontiguousarray(
            xs.reshape(KC, KCH, NITER, NB).transpose(2, 1, 0, 3))
        in_maps.append({
            "xt": xs, "weff": weff_d, "w2": w2_d, "w3": w3c,
            "b1": b1d, "b2": b2d,
        })
    return in_maps


def run(x, conv_w, w1, b1, w2, b2, w3, b3, trace=False):
    nc = _get_program()
    in_maps = make_in_maps(x, conv_w, w1, b1, w2, b2, w3, b3)
    br = run_bass_kernel_spmd(nc, in_maps, core_ids=list(range(N_CORES)),
                              trace=trace)
    out = np.empty((B, NOUT), np.float32)
    for c in range(N_CORES):
        # [NITER, 10, 512] -> [8192, 10]
        r = br.results[c]["out"]
        out[c * BS:(c + 1) * BS] = r.transpose(0, 2, 1).reshape(BS, NOUT)
    out += np.asarray(b3, np.float32)[None, :]
    return out, br


def kernel(x, conv_w, w1, b1, w2, b2, w3, b3):
    out, _ = run(x, conv_w, w1, b1, w2, b2, w3, b3)
    return out

